# revision 37
# baseline (speedup 1.0000x reference)
"""Multi-head attention (B=2,S=2048,E=1024,H=16,DK=DV=64) on 8 Trainium2 cores.

Sharding: core c handles batch c//4 and head-group c%4 (4 heads each).
Each core computes q/k/v projections for its heads, masked softmax attention
(transposed-scores layout), and a partial output projection with its rows of
Wo.  The host sums the 4 partial outputs per batch and adds bo.

The kernel is Scalar-engine bound (~147us of Exp at 1 elem/lane/cycle), so
the structure exists to start that stream early and keep it fed:
 - Scalar does ONLY exps (1/sqrt(DK) folded into the activation scale,
   biases added on DVE via tensor_scalar, all copies on DVE).
 - Weights ship as one [128, 8192] tensor (4KB lines; 512B lines DMA 4x
   slower).  xq/xk arrive in sequence-halves so the h0 projections (5.2MB
   in) unblock the first attention block ~40us in; h1 projections and the
   q0=0 output projection are deferred onto the st PSUM ring as inserts in
   later blocks (the PE executes in order, so anything between dependent
   attention steps must be independent work).
 - ctx matmuls are emitted 1-2 tiles behind their scores/exp so they never
   block the next scores (they wait on the exp; es ring buffers 12).
 - PSUM: st ring 4 banks + ctx 2x2 banks; proj pools use the remaining 4
   banks and close before the ctx pool opens.  Mask is staged as per-q0
   half tiles on a 22-slot ring; y is written back as bf16.
"""

import numpy as np
import ml_dtypes

import concourse.bacc as bacc
import concourse.mybir as mybir
import concourse.tile as tile
from concourse import bass_utils

BF = ml_dtypes.bfloat16
F8 = ml_dtypes.float8_e4m3
dt = mybir.dt

NCORES = 8


def _emit(nc, tc, inp, y_d, S, E, HL, DK):
    EC = E // 128          # contraction chunks for the projections
    NT = S // 128          # seq tiles
    DKL = HL * DK          # local head dims (256)
    NP = DKL // 128        # q/k partition tiles (pairs of heads)
    Exp = mybir.ActivationFunctionType.Exp
    CS = 512               # matmul free-dim chunk (one PSUM bank of fp32)
    QB = 1024              # query block (st tile = 2 banks)

    persist = tc.alloc_tile_pool(name="persist", bufs=1)
    qT = [persist.tile([128, S], dt.bfloat16, name=f"qT{m}") for m in range(NP)]
    kT = [persist.tile([128, S], dt.bfloat16, name=f"kT{m}") for m in range(NP)]
    cT = [persist.tile([128, S], dt.bfloat16, name=f"cT{m}") for m in range(NP)]
    VW = HL * (DK + 1)
    vAall = persist.tile([128, NT * VW], dt.bfloat16, name="vAall")
    vA = [vAall[:, t * VW:(t + 1) * VW] for t in range(NT)]

    # all weights in one tensor with 4KB+ per-partition lines (512B lines
    # DMA at ~1/4 speed): [wv | wq | wk | wo]
    WSEG = EC * DKL
    wall = persist.tile([128, 3 * WSEG + NP * E], dt.bfloat16, name="wall")
    nc.sync.dma_start(wall[:], inp["wall"][:])
    w_sb = {}
    for i, nm in enumerate(("wv", "wq", "wk")):
        w_sb[nm] = [wall[:, i * WSEG + c * DKL:i * WSEG + (c + 1) * DKL]
                    for c in range(EC)]
    wo_sb = [wall[:, 3 * WSEG + p * E:3 * WSEG + (p + 1) * E] for p in range(NP)]
    bqk = persist.tile([128, 2 * NP], dt.float32, name="bqk")
    nc.sync.dma_start(bqk[:], inp["bqk"][:])
    bv1 = persist.tile([1, DKL], dt.bfloat16, name="bv1")
    nc.sync.dma_start(bv1[:], inp["bv"][:])

    # mask in per-q0 half tiles on a ring: each half is consumed entirely
    # by the two adjacent hp-blocks of one q0 iteration, so 22 slots cover
    # the stream with no re-DMA (full tiles would pin 64KB/partition).
    mpool = tc.alloc_tile_pool(name="mask", bufs=1)
    mh = {}
    for q0i in range(S // QB):
        for t in range(NT):
            mh[(q0i, t)] = mpool.tile([128, QB], dt.bfloat16, tag="m",
                                      bufs=22, name=f"mask{q0i}_{t}")

    # preload the Exp activation-table while the input DMAs stream in
    dumm = persist.tile([1, 4], dt.float32, name="dumm")
    nc.gpsimd.memset(dumm[:], 0.0)
    dumo = persist.tile([1, 4], dt.float32, name="dumo")
    nc.scalar.activation(dumo[:], dumm[:], Exp)

    bvb = persist.tile([128, DKL], dt.bfloat16, name="bvb")
    nc.gpsimd.partition_broadcast(bvb[:], bv1[:])
    nc.gpsimd.memset(vAall[:], 1.0)

    # st ring allocated BEFORE the proj pools so it coexists with them
    # (PSUM: st 4 banks + qk 2 + v 2 = 8; ctx pool reuses qk+v space later).
    stpool = tc.alloc_tile_pool(name="stps", bufs=1, space="PSUM")
    npool = tc.alloc_tile_pool(name="nrm", bufs=1)
    ypool = tc.alloc_tile_pool(name="ysb", bufs=2)

    # ---- phase 1: projections ---------------------------------------------
    # x arrives in sequence-halves: h0 of xq/xk (5.2MB with weights) gates
    # the whole first attention block, so exp starts ~40us in; the h1-half
    # projections are deferred into the first two attention blocks.  xh1
    # lives in its own pool (late consumers) created BELOW the h0+v pool so
    # the latter can be released before the es ring allocates (LIFO).
    HB = S // 2
    xh1_pool = tc.alloc_tile_pool(name="xh1", bufs=1)
    xh1 = {nm: [xh1_pool.tile([128, HB], dt.bfloat16, tag="xh1", bufs=16,
                              name=f"{nm}h1_{c}") for c in range(EC)]
           for nm in ("q", "k")}
    xv_pool = tc.alloc_tile_pool(name="xvp", bufs=1)
    xv = [xv_pool.tile([128, S], dt.bfloat16, tag="xv", bufs=8,
                       name=f"xv{c}") for c in range(EC)]
    xa_pool = tc.alloc_tile_pool(name="xa", bufs=1)
    xh0 = {nm: [xa_pool.tile([128, HB], dt.bfloat16, tag="xh0", bufs=16,
                             name=f"{nm}h0_{c}") for c in range(EC)]
           for nm in ("q", "k")}

    for nm in ("q", "k"):
        for c in range(EC):
            nc.sync.dma_start(xh0[nm][c][:], inp["x" + nm][c][:, 0:HB])
    for c in range(EC):
        nc.sync.dma_start(xv[c][:], inp["xv"][c])
    for t in range(3):
        nc.sync.dma_start(mh[(0, t)][:], inp["mask"][:, t, 0:QB])
    for c in range(EC):
        nc.sync.dma_start(xh1["k"][c][:], inp["xk"][c][:, HB:S])
    for c in range(EC):
        nc.sync.dma_start(xh1["q"][c][:], inp["xq"][c][:, HB:S])
    for t in range(3, NT):
        nc.sync.dma_start(mh[(0, t)][:], inp["mask"][:, t, 0:QB])
    for t in range(NT):
        nc.sync.dma_start(mh[(1, t)][:], inp["mask"][:, t, QB:S])

    PROJ = {"q": (qT, 0), "k": (kT, NP)}

    with tc.tile_pool(name="qkps", bufs=2, space="PSUM") as qkps:
        for nm in ("q", "k"):
            dst, boff = PROJ[nm]
            for m in range(NP):
                for n0 in range(0, HB, CS):
                    ps = qkps.tile([128, CS], dt.float32, tag="qk",
                                   name=f"{nm}ps{m}_{n0}")
                    for c in range(EC):
                        nc.tensor.matmul(
                            ps[:], w_sb["w" + nm][c][:, 128 * m:128 * (m + 1)],
                            xh0[nm][c][:, n0:n0 + CS],
                            start=(c == 0), stop=(c == EC - 1))
                    nc.vector.tensor_scalar_add(
                        dst[m][:, n0:n0 + CS], ps[:],
                        bqk[:, boff + m:boff + m + 1])

    xa_pool.release()
    epool = tc.alloc_tile_pool(name="es", bufs=1)

    def _proj_group(nm, m, n0):
        # one deferred second-half projection chunk on the st PSUM ring
        dst, boff = PROJ[nm]
        ps = stpool.tile([128, QB], dt.float32, tag="st", bufs=2,
                         name=f"{nm}ps{m}_{n0}d")
        for c in range(EC):
            nc.tensor.matmul(
                ps[:, 0:CS], w_sb["w" + nm][c][:, 128 * m:128 * (m + 1)],
                xh1[nm][c][:, n0 - HB:n0 - HB + CS],
                start=(c == 0), stop=(c == EC - 1))
        nc.vector.tensor_scalar_add(
            dst[m][:, n0:n0 + CS], ps[:, 0:CS], bqk[:, boff + m:boff + m + 1])

    def _v_group(t):
        # one v-projection tile on the st PSUM ring (the only spare banks
        # during attention); DVE adds the bias and scatters into vA
        vps = stpool.tile([128, QB], dt.float32, tag="st", bufs=2,
                          name=f"vps{t}")
        for c in range(EC):
            nc.tensor.matmul(vps[:, 0:DKL],
                             xt["xv"][c][:, t * 128:(t + 1) * 128],
                             w_sb["wv"][c][:],
                             start=(c == 0), stop=(c == EC - 1))
        nc.vector.tensor_add(
            vA[t][:].rearrange("p (h c) -> p h c", h=HL)[:, :, 0:DK],
            vps[:, 0:DKL].rearrange("p (h c) -> p h c", h=HL),
            bvb[:].rearrange("p (h c) -> p h c", h=HL))

    def _v_group(t):
        # one v-projection tile on the st PSUM ring; DVE adds the bias
        vps = stpool.tile([128, QB], dt.float32, tag="st", bufs=2,
                          name=f"vps{t}")
        for c in range(EC):
            nc.tensor.matmul(vps[:, 0:DKL],
                             xv[c][:, t * 128:(t + 1) * 128],
                             w_sb["wv"][c][:],
                             start=(c == 0), stop=(c == EC - 1))
        nc.vector.tensor_add(
            vA[t][:].rearrange("p (h c) -> p h c", h=HL)[:, :, 0:DK],
            vps[:, 0:DKL].rearrange("p (h c) -> p h c", h=HL),
            bvb[:].rearrange("p (h c) -> p h c", h=HL))

    def _yproj_group(s):
        # one output-projection seq-tile on the st PSUM ring
        yp = stpool.tile([128, E], dt.float32, tag="st", bufs=2,
                         name=f"yp{s}")
        for p in range(NP):
            for e0 in range(0, E, CS):
                nc.tensor.matmul(yp[:, e0:e0 + CS],
                                 cT[p][:, s * 128:(s + 1) * 128],
                                 wo_sb[p][:, e0:e0 + CS],
                                 start=(p == 0), stop=(p == NP - 1))
        ysb = ypool.tile([128, E], dt.bfloat16, tag="y", bufs=2,
                         name=f"ysb{s}")
        nc.vector.tensor_copy(ysb[:], yp[:])
        nc.sync.dma_start(y_d[s * 128:(s + 1) * 128, :], ysb[:])

    # ---- phase 2: attention (2 interleaved chains) + fused yproj ----------
    # ctx matmuls are emitted one tile BEHIND the scores/exp of the same
    # chain: the PE executes its queue in order, so without the lag the ctx
    # (which waits on the exp) blocks the next tile's scores and the scalar
    # engine stalls ~0.8us per exp.  Pair-1 q/k projections are emitted as
    # filler into the first steps, using the ctx-pool PSUM rings before the
    # first real ctx tiles claim them.
    with tc.tile_pool(name="ctxps", bufs=1, space="PSUM") as ctxpool:
        ERING = 14
        for q0 in range(0, S, QB):
            for hp in range(NP):
                bi = 2 * (q0 // QB) + hp
                heads = (2 * hp, 2 * hp + 1)
                ctxs = {}
                pend = []          # (t, h, es) ctx matmuls not yet emitted
                first = (bi == 0)
                # deferred second-half projections ride the st ring in the
                # first two blocks (k halves before their t=8 use; q halves
                # before the q0=1024 blocks)
                if bi == 0:
                    inserts = {1 + 2 * i: (lambda i=i: _proj_group(
                        "k", i // 2, HB + CS * (i % 2))) for i in range(4)}
                elif bi == 1:
                    inserts = {1 + 2 * i: (lambda i=i: _proj_group(
                        "q", i // 2, HB + CS * (i % 2))) for i in range(4)}
                elif bi == 2:
                    # yproj of the previous q-block rides this block's steps
                    inserts = {1 + 2 * i: (lambda i=i: _yproj_group(i))
                               for i in range(8)}
                else:
                    inserts = {}

                def _ctx_flush(upto, limit=None):
                    ready = [p for p in pend if p[0] <= upto]
                    if limit is not None:
                        ready = ready[:limit]
                    for t_, h_, e_ in ready:
                        if h_ not in ctxs:
                            ctxs[h_] = ctxpool.tile(
                                [128, QB], dt.float32, tag=f"ctx{h_ % 2}",
                                name=f"ctx{h_}_{q0}")
                        for n0 in range(0, QB, CS):
                            nc.tensor.matmul(
                                ctxs[h_][0:DK + 1, n0:n0 + CS],
                                vA[t_][:, h_ * (DK + 1):(h_ + 1) * (DK + 1)],
                                e_[:, n0:n0 + CS],
                                start=(t_ == 0), stop=(t_ == NT - 1))
                        pend.remove((t_, h_, e_))

                for t in range(NT):
                    sts = {}
                    for n0 in range(0, QB, CS):
                        for ci, h in enumerate(heads):
                            sub = (h % 2) * 64
                            if n0 == 0:
                                sts[h] = stpool.tile(
                                    [128, QB], dt.float32, tag="st", bufs=2,
                                    name=f"st{h}_{t}_{q0}")
                            nc.tensor.matmul(
                                sts[h][:, n0:n0 + CS],
                                kT[hp][sub:sub + DK, t * 128:(t + 1) * 128],
                                qT[hp][sub:sub + DK, q0 + n0:q0 + n0 + CS],
                                start=True, stop=True)
                    for ci, h in enumerate(heads):
                        e = epool.tile([128, QB], dt.bfloat16, tag="e",
                                       bufs=ERING, name=f"e{h}_{t}_{q0}")
                        nc.scalar.activation(e[:], sts[h][:], Exp, scale=0.125)
                        nc.vector.tensor_mul(e[:], e[:], mt[t][:, q0:q0 + QB])
                        pend.append((t, h, e))
                    if t in inserts:
                        inserts[t]()
                    if first:
                        # v-projection rides the st ring, two tiles behind
                        if t >= 2:
                            _v_group(t - 2)
                        _ctx_flush(t - 4, limit=4)
                    elif bi == 1:
                        _ctx_flush(t - 2, limit=4)
                    else:
                        _ctx_flush(t - 1, limit=4)
                if first:
                    _v_group(NT - 2)
                    _v_group(NT - 1)
                _ctx_flush(NT - 1)
                for ci, h in enumerate(heads):
                    sub = (h % 2) * 64
                    ctx = ctxs[h]
                    dn = npool.tile([1, QB], dt.float32, tag="dn",
                                    name=f"dn{h}_{q0}")
                    nc.vector.tensor_copy(dn[:], ctx[DK:DK + 1, 0:QB])
                    bd = npool.tile([DK, QB], dt.float32, tag="bd",
                                    name=f"bd{h}_{q0}")
                    nc.gpsimd.partition_broadcast(bd[:], dn[:])
                    bc = npool.tile([DK, QB], dt.float32, tag="bc",
                                    name=f"bc{h}_{q0}")
                    nc.vector.reciprocal_approx_fast(bc[:], bd[:])
                    nc.vector.tensor_mul(cT[hp][sub:sub + DK, q0:q0 + QB],
                                         ctx[0:DK, 0:QB], bc[:])
            if q0 + QB >= S:
                for si in range(QB // 128):
                    _yproj_group(q0 // 128 + si)

    epool.release()
    xs_pool.release()
    ypool.release()
    npool.release()
    stpool.release()
    mpool.release()
    persist.release()


def _build(S, E, HL, DK):
    EC = E // 128
    NT = S // 128
    DKL = HL * DK
    NP = DKL // 128
    nc = bacc.Bacc("TRN2", target_bir_lowering=False, debug=False,
                   num_devices=NCORES)
    inp = {}
    for nm in ("xq", "xk", "xv"):
        inp[nm] = nc.dram_tensor(nm, [EC, 128, S], dt.bfloat16,
                                 kind="ExternalInput").ap()
    inp["wall"] = nc.dram_tensor("wall", [128, 3 * EC * DKL + NP * E],
                                 dt.bfloat16, kind="ExternalInput").ap()
    inp["bqk"] = nc.dram_tensor("bqk", [128, 2 * NP], dt.float32,
                                kind="ExternalInput").ap()
    inp["bv"] = nc.dram_tensor("bv", [1, DKL], dt.bfloat16,
                               kind="ExternalInput").ap()
    inp["mask"] = nc.dram_tensor("mask", [128, NT, S], dt.bfloat16,
                                 kind="ExternalInput").ap()
    y_d = nc.dram_tensor("y", [S, E], dt.bfloat16, kind="ExternalOutput").ap()

    with tile.TileContext(nc) as tc:
        _emit(nc, tc, inp, y_d, S, E, HL, DK)
    nc.compile()
    return nc


_CACHE = {}
_TRACE = False
_TRACE_CORES = (0,)
_LAST_RESULT = None


def _get_nc(S, E, HL, DK):
    key = (S, E, HL, DK)
    if key not in _CACHE:
        _CACHE[key] = _build(S, E, HL, DK)
    return _CACHE[key]


_RUNNER_CACHE = {}


def _get_runner(nc):
    """Cached variant of bass2jax.run_bass_via_pjrt's multi-core path: build
    the jitted shard_map executable once and reuse it across kernel() calls
    (a fresh jax.jit per call re-traces and may recompile)."""
    if id(nc) in _RUNNER_CACHE:
        return _RUNNER_CACHE[id(nc)]
    import jax
    import concourse.mybir as _mybir
    from concourse import bass2jax
    from jax.sharding import Mesh, PartitionSpec
    from jax.experimental.shard_map import shard_map

    bass2jax.install_neuronx_cc_hook()
    pid_name = nc.partition_id_tensor.name if nc.partition_id_tensor else None
    in_names, out_names, out_avals, zero_shapes = [], [], [], []
    for alloc in nc.m.functions[0].allocations:
        if not isinstance(alloc, _mybir.MemoryLocationSet):
            continue
        name = alloc.memorylocations[0].name
        if alloc.kind == "ExternalInput":
            if name != pid_name:
                in_names.append(name)
        elif alloc.kind == "ExternalOutput":
            out_names.append(name)
            shape = tuple(alloc.tensor_shape)
            dtype = _mybir.dt.np(alloc.dtype)
            out_avals.append(jax.core.ShapedArray(shape, dtype))
            zero_shapes.append((shape, dtype))
    n_params = len(in_names)
    n_outs = len(out_avals)
    all_names = in_names + out_names
    if pid_name is not None:
        all_names = all_names + [pid_name]

    def _body(*args):
        operands = list(args)
        if pid_name is not None:
            operands.append(bass2jax.partition_id_tensor())
        return tuple(bass2jax._bass_exec_p.bind(
            *operands,
            out_avals=tuple(out_avals),
            in_names=tuple(all_names),
            out_names=tuple(out_names),
            lowering_input_output_aliases=(),
            sim_require_finite=True,
            sim_require_nnan=True,
            nc=nc,
        ))

    devices = jax.devices()[:NCORES]
    mesh = Mesh(np.asarray(devices), ("core",))
    donate = tuple(range(n_params, n_params + n_outs))
    sharded = jax.jit(
        shard_map(_body, mesh=mesh,
                  in_specs=(PartitionSpec("core"),) * (n_params + n_outs),
                  out_specs=(PartitionSpec("core"),) * n_outs,
                  check_rep=False),
        donate_argnums=donate, keep_unused=True)

    def run(in_maps):
        concat_in = [np.concatenate([np.asarray(m[nm]) for m in in_maps], axis=0)
                     for nm in in_names]
        concat_zeros = [np.zeros((NCORES * s[0], *s[1:]), d)
                        for s, d in zero_shapes]
        outs = sharded(*concat_in, *concat_zeros)
        return [
            {nm: np.asarray(outs[i]).reshape(NCORES, *out_avals[i].shape)[c]
             for i, nm in enumerate(out_names)}
            for c in range(NCORES)
        ]

    _RUNNER_CACHE[id(nc)] = run
    return run


def run_sharded(query, key, value, mask, Wq, bq, Wk, bk, Wv, bv, Wo, bo):
    """Full-input -> full-output runner (generic shapes)."""
    global _LAST_RESULT
    query, key, value = (np.asarray(a, np.float32) for a in (query, key, value))
    mask = np.asarray(mask)
    Wq, bq, Wk, bk, Wv, bv, Wo, bo = (
        np.asarray(a, np.float32) for a in (Wq, bq, Wk, bk, Wv, bv, Wo, bo))

    B, S, E = query.shape
    HDK = Wq.shape[1]
    DKv = 64
    H = HDK // DKv
    GPB = NCORES // B                 # cores per batch
    HL = H // GPB                     # heads per core
    DKL = HL * DKv
    NP = DKL // 128
    EC = E // 128
    NT = S // 128

    nc = _get_nc(S, E, HL, DKv)

    # per-batch host prep (shared by the 4 cores of a batch)
    xb = {}
    for b in range(B):
        xb[b] = {
            "xq": np.ascontiguousarray(query[b].T).astype(BF).reshape(EC, 128, S),
            "xk": np.ascontiguousarray(key[b].T).astype(BF).reshape(EC, 128, S),
            "xv": np.ascontiguousarray(value[b].T).astype(BF).reshape(EC, 128, S),
            "mask": np.ascontiguousarray(
                mask[b].reshape(S, NT, 128).transpose(2, 1, 0)).astype(BF),
        }

    in_maps = []
    for c in range(NCORES):
        b, g = c // GPB, c % GPB
        sl = slice(g * DKL, (g + 1) * DKL)
        bqk = np.concatenate([bq[sl].reshape(NP, 128).T,
                              bk[sl].reshape(NP, 128).T], axis=1)
        # wall: [wv | wq | wk | wo] with 128-partition-major big lines
        segs = [W[:, sl].reshape(EC, 128, DKL).transpose(1, 0, 2).reshape(
                    128, EC * DKL) for W in (Wv, Wq, Wk)]
        segs.append(Wo[sl, :].reshape(NP, 128, E).transpose(1, 0, 2).reshape(
            128, NP * E))
        in_maps.append({
            **xb[b],
            "wall": np.ascontiguousarray(np.concatenate(segs, axis=1)).astype(BF),
            "bqk": np.ascontiguousarray(bqk).astype(np.float32),
            "bv": bv[sl].astype(BF).reshape(1, DKL),
        })

    if _TRACE:
        res = bass_utils.run_bass_kernel_spmd(
            nc, in_maps, core_ids=list(range(NCORES)),
            trace=True, trace_cores=list(_TRACE_CORES))
        _LAST_RESULT = res
        results = res.results
    else:
        results = _get_runner(nc)(in_maps)

    y = np.zeros((B, S, E), np.float32)
    for c in range(NCORES):
        y[c // GPB] += results[c]["y"].astype(np.float32)
    y += bo.astype(np.float32)
    return y


def kernel(**inputs):
    return run_sharded(
        inputs["query"], inputs["key"], inputs["value"], inputs["mask"],
        inputs["Wq"], inputs["bq"], inputs["Wk"], inputs["bk"],
        inputs["Wv"], inputs["bv"], inputs["Wo"], inputs["bo"])
